# revision 16
# baseline (speedup 1.0000x reference)
"""Trainium2 Bass kernel for nn_FANPhaseOffsetTransformerLayer.

Full inputs -> full output. Sharding: 8 cores; core c handles batch b=c//4
and sequence-row chunk qc=c%4 (512 rows) of that batch. Each core computes
k/v for its whole batch (4x redundant, zero communication), q only for its
row chunk, attention for its rows over all 16 heads, then Wo/LN1/FAN/LN2
for its rows. Host gathers the 8 row-chunks into the full output.

Matmuls run in bf16 (fp32 PSUM accumulation); softmax scores and LN math
in fp32. Softmax skips the max-subtraction (scores are bounded ~+-4 here,
exp is safe in fp32), and the denominator is produced by a 65th ones-column
appended to v in the PV matmul.
"""

import math

import numpy as np
import ml_dtypes

B, S, D, H, E = 2, 2048, 1024, 16, 64
P_DIM, G_DIM = 256, 512
SC = 512  # rows per core
NCORES = 8
LN_EPS = 1e-5

_bf = ml_dtypes.bfloat16

_prog_cache = {}


def _build_program(gv: float):
    from contextlib import ExitStack

    import concourse.bass as bass
    import concourse.bacc as bacc
    import concourse.tile as tile
    import concourse.mybir as mybir

    f32 = mybir.dt.float32
    bf = mybir.dt.bfloat16
    f32r = mybir.dt.float32r
    AF = mybir.ActivationFunctionType
    ALU = mybir.AluOpType

    nc = bacc.Bacc(
        "TRN2",
        target_bir_lowering=False,
        debug=False,
        enable_asserts=False,
        num_devices=NCORES,
    )

    # ---------------- DRAM I/O ----------------
    d_xqT = nc.dram_tensor("xqT", [D, SC], bf, kind="ExternalInput")
    cc_kin = nc.dram_tensor("cc_kin", [D * SC], bf, kind="Internal")
    cc_vin = nc.dram_tensor("cc_vin", [SC * D], bf, kind="Internal")
    cc_kout = nc.dram_tensor("cc_kout", [4, D * SC], bf, kind="Internal")
    cc_vout = nc.dram_tensor("cc_vout", [4, SC * D], bf, kind="Internal")
    d_xres = nc.dram_tensor("xres", [SC, D], f32, kind="ExternalInput")
    d_wqT = nc.dram_tensor("wqT", [D, D], bf, kind="ExternalInput")
    d_wkT = nc.dram_tensor("wkT", [D, D], bf, kind="ExternalInput")
    d_wvT = nc.dram_tensor("wvT", [D, D], bf, kind="ExternalInput")
    d_woT = nc.dram_tensor("woT", [D, D], bf, kind="ExternalInput")
    d_wpT = nc.dram_tensor("wpT", [D, P_DIM], bf, kind="ExternalInput")
    d_wgT = nc.dram_tensor("wgT", [D, G_DIM], bf, kind="ExternalInput")
    d_bqc = nc.dram_tensor("bqc", [128, 8], f32, kind="ExternalInput")
    d_bkc = nc.dram_tensor("bkc", [128, 8], f32, kind="ExternalInput")
    d_bvr = nc.dram_tensor("bvr", [1, D], bf, kind="ExternalInput")
    d_bor = nc.dram_tensor("bor", [1, D], bf, kind="ExternalInput")
    d_bpr = nc.dram_tensor("bpr", [1, P_DIM], bf, kind="ExternalInput")
    d_bgr = nc.dram_tensor("bgr", [1, G_DIM], bf, kind="ExternalInput")
    d_ln1w = nc.dram_tensor("ln1w", [D], f32, kind="ExternalInput")
    d_ln1b = nc.dram_tensor("ln1b", [D], f32, kind="ExternalInput")
    d_ln2w = nc.dram_tensor("ln2w", [D], f32, kind="ExternalInput")
    d_ln2b = nc.dram_tensor("ln2b", [D], f32, kind="ExternalInput")
    d_offs = nc.dram_tensor("offs", [P_DIM], f32, kind="ExternalInput")
    d_offc = nc.dram_tensor("offc", [P_DIM], f32, kind="ExternalInput")
    d_sel = nc.dram_tensor("sel", [16, 16, 64], f32r, kind="ExternalInput")
    d_ident = nc.dram_tensor("ident", [128, 128], f32, kind="ExternalInput")
    d_out = nc.dram_tensor("out", [SC, D], f32, kind="ExternalOutput")

    def bcast(handle, parts):
        ap_ = handle.ap()
        return bass.AP(
            tensor=ap_.tensor, offset=ap_.offset, ap=[[0, parts]] + list(ap_.ap)
        )

    with tile.TileContext(nc, pool_alloc_mode="queue") as tc:
        with ExitStack() as ctx:
            misc1 = tc.alloc_tile_pool(name="misc1", bufs=1)
            kv = tc.alloc_tile_pool(name="kv", bufs=1, side="right")

            # ------- small constants needed in QKV phase -------
            bqc_sb = misc1.tile([128, 8], f32)
            nc.gpsimd.dma_start(out=bqc_sb, in_=d_bqc.ap())
            bkc_sb = misc1.tile([128, 8], f32)
            nc.gpsimd.dma_start(out=bkc_sb, in_=d_bkc.ap())
            bvr_sb = misc1.tile([1, D], bf)
            nc.gpsimd.dma_start(out=bvr_sb, in_=d_bvr.ap())
            bor_sb = misc1.tile([1, D], bf)
            nc.gpsimd.dma_start(out=bor_sb, in_=d_bor.ap())
            bpr_sb = misc1.tile([1, P_DIM], bf)
            nc.gpsimd.dma_start(out=bpr_sb, in_=d_bpr.ap())
            bgr_sb = misc1.tile([1, G_DIM], bf)
            nc.gpsimd.dma_start(out=bgr_sb, in_=d_bgr.ap())
            ones_row = misc1.tile([1, 128], bf)
            nc.vector.memset(ones_row, 1.0)
            eps_sb = misc1.tile([128, 1], f32)
            nc.vector.memset(eps_sb, LN_EPS)

            # ------- kv-phase persistent tiles -------
            qT_sb = kv.tile([128, 8, SC], bf)
            kT_sb = kv.tile([128, 8, S], bf)
            vaug = kv.tile([128, 16, 16, 65], bf)
            nc.vector.memset(vaug[:, :, :, 64:65], 1.0)

            # ================= QKV (own chunk) + AllGather + attention =================
            apo = tc.alloc_tile_pool(name="attnp", bufs=1, side="right")
            raw_sb = apo.tile([128, 16, 512], bf)
            den16 = apo.tile([16, 512], bf)
            rec16 = apo.tile([16, 512], f32r)

            RG = [[0, 1, 2, 3], [4, 5, 6, 7]]

            with tc.tile_pool(name="qkvw", bufs=1) as qkvw, tc.tile_pool(
                name="ppq", bufs=1, space="PSUM"
            ) as ppq:
                wq_sb = qkvw.tile([128, 8, D], bf)
                wk_sb = qkvw.tile([128, 8, D], bf)
                wv_sb = qkvw.tile([128, 8, D], bf)
                for kc in range(8):
                    nc.sync.dma_start(
                        out=wk_sb[:, kc, :], in_=d_wkT.ap()[kc * 128 : (kc + 1) * 128, :]
                    )
                    nc.sync.dma_start(
                        out=wv_sb[:, kc, :], in_=d_wvT.ap()[kc * 128 : (kc + 1) * 128, :]
                    )
                    nc.sync.dma_start(
                        out=wq_sb[:, kc, :], in_=d_wqT.ap()[kc * 128 : (kc + 1) * 128, :]
                    )
                xq_sb = qkvw.tile([128, 8, SC], bf)
                for kc in range(8):
                    nc.sync.dma_start(
                        out=xq_sb[:, kc, :], in_=d_xqT.ap()[kc * 128 : (kc + 1) * 128, :]
                    )

                # k^T for own chunk -> cc_kin, then AllGather
                kchunk = qkvw.tile([128, 8, SC], bf)
                for m in range(8):
                    ps = ppq.tile([128, SC], f32, tag="qkvps", bufs=2, name="kps")
                    for kc in range(8):
                        nc.tensor.matmul(
                            ps,
                            lhsT=wk_sb[:, kc, m * 128 : (m + 1) * 128],
                            rhs=xq_sb[:, kc, :],
                            start=(kc == 0),
                            stop=(kc == 7),
                        )
                    nc.vector.tensor_scalar(
                        out=kchunk[:, m, :],
                        in0=ps,
                        scalar1=bkc_sb[:, m : m + 1],
                        scalar2=None,
                        op0=ALU.add,
                    )
                    nc.sync.dma_start(
                        out=cc_kin.ap().rearrange("(m p n) -> m p n", m=8, p=128)[m],
                        in_=kchunk[:, m, :],
                    )
                nc.gpsimd.collective_compute(
                    "AllGather",
                    mybir.AluOpType.bypass,
                    replica_groups=RG,
                    ins=[cc_kin.ap()],
                    outs=[cc_kout.ap()],
                )

                # v for own chunk -> cc_vin, then AllGather
                vchunk = qkvw.tile([128, 4, D], bf)
                for tm in range(4):
                    for h2 in range(2):
                        ps = ppq.tile([128, 512], f32, tag="qkvps", bufs=2, name="vps")
                        for kc in range(8):
                            nc.tensor.matmul(
                                ps,
                                lhsT=xq_sb[:, kc, tm * 128 : (tm + 1) * 128],
                                rhs=wv_sb[:, kc, h2 * 512 : (h2 + 1) * 512],
                                start=(kc == 0),
                                stop=False,
                            )
                        nc.tensor.matmul(
                            ps,
                            lhsT=ones_row,
                            rhs=bvr_sb[:, h2 * 512 : (h2 + 1) * 512],
                            start=False,
                            stop=True,
                        )
                        nc.vector.tensor_copy(
                            vchunk[:, tm, h2 * 512 : (h2 + 1) * 512], ps
                        )
                    nc.sync.dma_start(
                        out=cc_vin.ap().rearrange("(t p n) -> t p n", t=4, p=128)[tm],
                        in_=vchunk[:, tm, :],
                    )
                nc.gpsimd.collective_compute(
                    "AllGather",
                    mybir.AluOpType.bypass,
                    replica_groups=RG,
                    ins=[cc_vin.ap()],
                    outs=[cc_vout.ap()],
                )

                # q^T for own chunk (overlaps the collectives)
                for m in range(8):
                    ps = ppq.tile([128, SC], f32, tag="qkvps", bufs=2, name="qps")
                    for kc in range(8):
                        nc.tensor.matmul(
                            ps,
                            lhsT=wq_sb[:, kc, m * 128 : (m + 1) * 128],
                            rhs=xq_sb[:, kc, :],
                            start=(kc == 0),
                            stop=(kc == 7),
                        )
                    nc.vector.tensor_scalar(
                        out=qT_sb[:, m, :],
                        in0=ps,
                        scalar1=bqc_sb[:, m : m + 1],
                        scalar2=None,
                        op0=ALU.add,
                    )

            with tc.tile_pool(name="ppa", bufs=1, space="PSUM") as ppa:
                # gathered k^T -> kT_sb   (rank r supplies t-cols [512r, 512r+512))
                ko = cc_kout.ap().rearrange("r (m p n) -> r m p n", m=8, p=128)
                for m in range(8):
                    for r in range(4):
                        nc.sync.dma_start(
                            out=kT_sb[:, m, r * 512 : (r + 1) * 512], in_=ko[r, m]
                        )
                # gathered v -> vaug
                vo = cc_vout.ap().rearrange("r (t p n) -> r t p n", t=4, p=128)
                for r in range(4):
                    for tm in range(4):
                        nc.sync.dma_start(
                            out=vaug[:, r * 4 + tm, :, 0:64], in_=vo[r, tm]
                        )

                def attn_pair(p):
                    opsums = []
                    for j in range(2):
                        op = ppa.tile(
                            [65, 512], f32, tag=f"opsum{j}", bufs=2, name=f"opsum{j}"
                        )
                        opsums.append(op)
                    for tb in range(16):
                        ps2 = ppa.tile(
                            [128, 2, 512], f32, tag="ps2", bufs=2, name="ps2"
                        )
                        for j in range(2):
                            off = j * 64
                            nc.tensor.matmul(
                                ps2[:, j, :],
                                lhsT=kT_sb[
                                    off : off + 64, p, tb * 128 : (tb + 1) * 128
                                ],
                                rhs=qT_sb[off : off + 64, p, :],
                                start=True,
                                stop=True,
                            )
                        probs = apo.tile(
                            [128, 2, 512], bf, tag="probs", bufs=3, name="probs"
                        )
                        nc.scalar.activation(
                            out=probs, in_=ps2, func=AF.Exp, scale=1.0 / math.sqrt(E)
                        )
                        for j in range(2):
                            nc.tensor.matmul(
                                opsums[j],
                                lhsT=vaug[:, tb, 2 * p + j, :],
                                rhs=probs[:, j, :],
                                start=(tb == 0),
                                stop=(tb == 15),
                            )
                    for j in range(2):
                        nc.vector.tensor_copy(raw_sb[0:65, 2 * p + j, :], opsums[j])

                for p in range(8):
                    attn_pair(p)

            # ------- post-phase constants (allocated after QKV pools free) -------
            misc2 = tc.alloc_tile_pool(name="misc2", bufs=1)
            ln1w_bc = misc2.tile([128, D], f32)
            nc.gpsimd.dma_start(out=ln1w_bc, in_=bcast(d_ln1w, 128))
            ln1b_bc = misc2.tile([128, D], f32)
            nc.gpsimd.dma_start(out=ln1b_bc, in_=bcast(d_ln1b, 128))
            ln2w_bc = misc2.tile([128, D], f32)
            nc.gpsimd.dma_start(out=ln2w_bc, in_=bcast(d_ln2w, 128))
            ln2b_bc = misc2.tile([128, D], f32)
            nc.gpsimd.dma_start(out=ln2b_bc, in_=bcast(d_ln2b, 128))
            offs_bc = misc2.tile([128, P_DIM], f32)
            nc.gpsimd.dma_start(out=offs_bc, in_=bcast(d_offs, 128))
            offc_bc = misc2.tile([128, P_DIM], f32)
            nc.gpsimd.dma_start(out=offc_bc, in_=bcast(d_offc, 128))
            sel_sb = misc2.tile([16, 16, 64], f32r)
            nc.gpsimd.dma_start(out=sel_sb, in_=d_sel.ap())
            ident_sb = misc2.tile([128, 128], f32)
            nc.gpsimd.dma_start(out=ident_sb, in_=d_ident.ap())
            xres_sb = misc2.tile([128, 4, D], f32)
            for sc in range(4):
                nc.sync.dma_start(
                    out=xres_sb[:, sc, :], in_=d_xres.ap()[sc * 128 : (sc + 1) * 128, :]
                )
            wo_sb = misc2.tile([128, 8, D], bf)
            for kc in range(8):
                nc.sync.dma_start(
                    out=wo_sb[:, kc, :], in_=d_woT.ap()[kc * 128 : (kc + 1) * 128, :]
                )
            attn_oT = misc2.tile([128, 8, SC], bf)
            odd_sb = misc2.tile([64, 8, 512], bf)

            # softmax denominators -> reciprocals
            nc.sync.dma_start(out=den16, in_=raw_sb[64:65, :, :])
            with nc.allow_low_precision(reason="softmax denominators: f32r rounding is ample"):
                nc.vector.reciprocal(rec16, den16)

            # normalize; even heads direct, odd heads staged then shifted
            with tc.tile_pool(name="ppn", bufs=2, space="PSUM") as ppn:
                for h in range(16):
                    p_, j = h // 2, h % 2
                    div = ppn.tile([64, 512], f32, tag="div", name="div")
                    nc.tensor.matmul(
                        div,
                        lhsT=sel_sb[:, h, :],
                        rhs=rec16,
                        start=True,
                        stop=True,
                    )
                    if j == 0:
                        out_ap = attn_oT[0:64, p_, :]
                    else:
                        out_ap = odd_sb[0:64, p_, :]
                    nc.vector.tensor_tensor(
                        out=out_ap, in0=raw_sb[0:64, h, :], in1=div, op=ALU.mult
                    )
            nc.sync.dma_start(out=attn_oT[64:128, :, :], in_=odd_sb)
            apo.release()
            kv.release()


            # ================= Wo + LN1 + FAN + LN2 =================
            with tc.tile_pool(name="pw", bufs=1) as pw, tc.tile_pool(
                name="post", bufs=2
            ) as po, tc.tile_pool(name="ppp", bufs=2, space="PSUM") as ppp:
                wp_sb = pw.tile([128, 8, P_DIM], bf)
                wg_sb = pw.tile([128, 8, G_DIM], bf)
                for kc in range(8):
                    nc.sync.dma_start(
                        out=wp_sb[:, kc, :], in_=d_wpT.ap()[kc * 128 : (kc + 1) * 128, :]
                    )
                    nc.sync.dma_start(
                        out=wg_sb[:, kc, :], in_=d_wgT.ap()[kc * 128 : (kc + 1) * 128, :]
                    )
                z_sb = pw.tile([128, 4, D], f32, tag="zfan", name="z_sb")
                y_sb = pw.tile([128, 4, D], f32)
                yT_sb = pw.tile([128, 8, SC], bf)
                fan_sb = pw.tile([128, 4, D], f32, tag="zfan", name="fan_sb")
                targ_sb = pw.tile([128, 4, 512], f32)
                g_sb = pw.tile([128, 4, 512], f32)

                def ln_apply(z_ap, w_bc, b_bc, out_ap):
                    stats = po.tile([128, 2, 6], f32, tag="lnst", name="lnst")
                    nc.vector.bn_stats(out=stats[:, 0, :], in_=z_ap[:, 0:512])
                    nc.vector.bn_stats(out=stats[:, 1, :], in_=z_ap[:, 512:1024])
                    mv = po.tile([128, 2], f32, tag="lnmv", name="lnmv")
                    nc.vector.bn_aggr(out=mv, in_=stats)
                    sd = po.tile([128, 2], f32, tag="lnsd", name="lnsd")
                    nc.scalar.activation(
                        out=sd[:, 0:1], in_=mv[:, 1:2], func=AF.Sqrt, bias=eps_sb
                    )
                    nc.vector.reciprocal(sd[:, 1:2], sd[:, 0:1])
                    tmp = po.tile([128, D], f32, tag="lntmp", name="lntmp")
                    nc.vector.tensor_scalar(
                        out=tmp,
                        in0=z_ap,
                        scalar1=mv[:, 0:1],
                        scalar2=sd[:, 1:2],
                        op0=ALU.subtract,
                        op1=ALU.mult,
                    )
                    nc.vector.tensor_tensor(out=tmp, in0=tmp, in1=w_bc, op=ALU.mult)
                    nc.vector.tensor_tensor(out=out_ap, in0=tmp, in1=b_bc, op=ALU.add)

                # Wo projection + residual
                for sc in range(4):
                    for h2 in range(2):
                        ps = ppp.tile([128, 512], f32, tag="wops", name="wops")
                        for kc in range(8):
                            nc.tensor.matmul(
                                ps,
                                lhsT=attn_oT[:, kc, sc * 128 : (sc + 1) * 128],
                                rhs=wo_sb[:, kc, h2 * 512 : (h2 + 1) * 512],
                                start=(kc == 0),
                                stop=False,
                            )
                        nc.tensor.matmul(
                            ps,
                            lhsT=ones_row,
                            rhs=bor_sb[:, h2 * 512 : (h2 + 1) * 512],
                            start=False,
                            stop=True,
                        )
                        nc.vector.tensor_tensor(
                            out=z_sb[:, sc, h2 * 512 : (h2 + 1) * 512],
                            in0=ps,
                            in1=xres_sb[:, sc, h2 * 512 : (h2 + 1) * 512],
                            op=ALU.add,
                        )
                for sc in range(4):
                    ln_apply(z_sb[:, sc, :], ln1w_bc, ln1b_bc, y_sb[:, sc, :])

                # transpose y
                for sc in range(4):
                    for dc in range(8):
                        tp = ppp.tile([128, 128], f32, tag="tp", name="tp")
                        nc.tensor.transpose(
                            tp, y_sb[:, sc, dc * 128 : (dc + 1) * 128], ident_sb
                        )
                        nc.vector.tensor_copy(
                            yT_sb[:, dc, sc * 128 : (sc + 1) * 128], tp
                        )

                # FAN matmuls + activation args
                for sc in range(4):
                    psp = ppp.tile([128, P_DIM], f32, tag="pps", name="pps")
                    for kc in range(8):
                        nc.tensor.matmul(
                            psp,
                            lhsT=yT_sb[:, kc, sc * 128 : (sc + 1) * 128],
                            rhs=wp_sb[:, kc, :],
                            start=(kc == 0),
                            stop=False,
                        )
                    nc.tensor.matmul(
                        psp, lhsT=ones_row, rhs=bpr_sb, start=False, stop=True
                    )
                    nc.vector.tensor_tensor(
                        out=targ_sb[:, sc, 0:256], in0=psp, in1=offs_bc, op=ALU.add
                    )
                    nc.vector.tensor_tensor(
                        out=targ_sb[:, sc, 256:512], in0=psp, in1=offc_bc, op=ALU.add
                    )
                    psg = ppp.tile([128, G_DIM], f32, tag="ppg", name="ppg")
                    for kc in range(8):
                        nc.tensor.matmul(
                            psg,
                            lhsT=yT_sb[:, kc, sc * 128 : (sc + 1) * 128],
                            rhs=wg_sb[:, kc, :],
                            start=(kc == 0),
                            stop=False,
                        )
                    nc.tensor.matmul(
                        psg, lhsT=ones_row, rhs=bgr_sb, start=False, stop=True
                    )
                    nc.vector.tensor_copy(g_sb[:, sc, :], psg)

                # range-reduce sin args to [-pi, pi]:
                # n = round(t/(2pi)) via the fp32 rounding constant; t -= 2pi*n
                RC = 12582912.0  # 1.5 * 2**23
                INV2PI = 1.0 / (2.0 * math.pi)
                for sc in range(4):
                    nred = po.tile([128, 512], f32, tag="nred", name="nred")
                    nc.vector.tensor_scalar(
                        out=nred,
                        in0=targ_sb[:, sc, :],
                        scalar1=INV2PI,
                        scalar2=RC,
                        op0=ALU.mult,
                        op1=ALU.add,
                    )
                    nc.vector.tensor_scalar(
                        out=nred,
                        in0=nred,
                        scalar1=RC,
                        scalar2=None,
                        op0=ALU.subtract,
                    )
                    nc.vector.scalar_tensor_tensor(
                        out=targ_sb[:, sc, :],
                        in0=nred,
                        scalar=-2.0 * math.pi,
                        in1=targ_sb[:, sc, :],
                        op0=ALU.mult,
                        op1=ALU.add,
                    )

                # batched activations (one table set each)
                for sc in range(4):
                    nc.scalar.activation(
                        out=targ_sb[:, sc, :], in_=targ_sb[:, sc, :], func=AF.Sin
                    )
                for sc in range(4):
                    nc.scalar.activation(
                        out=g_sb[:, sc, :], in_=g_sb[:, sc, :], func=AF.Gelu
                    )
                for sc in range(4):
                    nc.vector.tensor_scalar(
                        out=fan_sb[:, sc, 0:512],
                        in0=targ_sb[:, sc, :],
                        scalar1=float(gv),
                        scalar2=None,
                        op0=ALU.mult,
                    )
                    nc.vector.tensor_scalar(
                        out=fan_sb[:, sc, 512:1024],
                        in0=g_sb[:, sc, :],
                        scalar1=float(1.0 - gv),
                        scalar2=None,
                        op0=ALU.mult,
                    )

                # LN2 + output
                for sc in range(4):
                    z2 = po.tile([128, D], f32, tag="z2", name="z2")
                    nc.vector.tensor_tensor(
                        out=z2, in0=y_sb[:, sc, :], in1=fan_sb[:, sc, :], op=ALU.add
                    )
                    outt = po.tile([128, D], f32, tag="outt", name="outt")
                    ln_apply(z2, ln2w_bc, ln2b_bc, outt)
                    nc.sync.dma_start(
                        out=d_out.ap()[sc * 128 : (sc + 1) * 128, :], in_=outt
                    )

            misc2.release()
            misc1.release()

    nc.compile()
    return nc


def _host_inputs(inputs):
    """Build the per-core in_maps (list of 8 dicts) plus baked gate value."""
    f32 = np.float32
    x = np.asarray(inputs["x"], f32)
    Wq = np.asarray(inputs["Wq"], f32)
    Wk = np.asarray(inputs["Wk"], f32)
    Wv = np.asarray(inputs["Wv"], f32)
    Wo = np.asarray(inputs["Wo"], f32)
    Wp = np.asarray(inputs["Wp"], f32)
    Wg = np.asarray(inputs["Wg"], f32)
    bq = np.asarray(inputs["bq"], f32)
    bk = np.asarray(inputs["bk"], f32)
    bv = np.asarray(inputs["bv"], f32)
    bo = np.asarray(inputs["bo"], f32)
    bp = np.asarray(inputs["bp"], f32)
    bg = np.asarray(inputs["bg"], f32)
    offset = np.asarray(inputs["offset"], f32)
    gate = np.asarray(inputs["gate"], f32)
    ln1_w = np.asarray(inputs["ln1_w"], f32)
    ln1_b = np.asarray(inputs["ln1_b"], f32)
    ln2_w = np.asarray(inputs["ln2_w"], f32)
    ln2_b = np.asarray(inputs["ln2_b"], f32)

    gv = float(1.0 / (1.0 + np.exp(-gate[0])))

    sel = np.zeros((16, 16, 64), f32)
    for h in range(16):
        sel[h, h, :] = 1.0
    ident = np.eye(128, dtype=f32)

    shared = {
        "wqT": np.ascontiguousarray(Wq.T).astype(_bf),
        "wkT": np.ascontiguousarray(Wk.T).astype(_bf),
        "wvT": np.ascontiguousarray(Wv.T).astype(_bf),
        "woT": np.ascontiguousarray(Wo.T).astype(_bf),
        "wpT": np.ascontiguousarray(Wp.T).astype(_bf),
        "wgT": np.ascontiguousarray(Wg.T).astype(_bf),
        "bqc": np.ascontiguousarray(bq.reshape(8, 128).T),
        "bkc": np.ascontiguousarray(bk.reshape(8, 128).T),
        "bvr": bv.reshape(1, D).astype(_bf),
        "bor": bo.reshape(1, D).astype(_bf),
        "bpr": bp.reshape(1, P_DIM).astype(_bf),
        "bgr": bg.reshape(1, G_DIM).astype(_bf),
        "ln1w": ln1_w,
        "ln1b": ln1_b,
        "ln2w": ln2_w,
        "ln2b": ln2_b,
        "offs": offset,
        "offc": (np.pi - offset).astype(f32),
        "sel": sel,
        "ident": ident,
    }

    in_maps = []
    for c in range(NCORES):
        b, qc = c // 4, c % 4
        m = dict(shared)
        m["xqT"] = np.ascontiguousarray(
            x[b, qc * SC : (qc + 1) * SC].T
        ).astype(_bf)
        m["xres"] = np.ascontiguousarray(x[b, qc * SC : (qc + 1) * SC])
        in_maps.append(m)
    return in_maps, gv


def run(inputs, trace=False, tmpdir=None):
    """Run the kernel; returns (full_output, BassKernelResults)."""
    from concourse.bass_utils import run_bass_kernel_spmd

    in_maps, gv = _host_inputs(inputs)
    key = round(gv, 9)
    if key not in _prog_cache:
        _prog_cache[key] = _build_program(gv)
    nc = _prog_cache[key]
    res = run_bass_kernel_spmd(
        nc, in_maps, core_ids=list(range(NCORES)), trace=trace, tmpdir=tmpdir
    )
    chunks = [res.results[c]["out"] for c in range(NCORES)]
    full = np.concatenate(chunks, axis=0).reshape(B, S, D).astype(np.float32)
    return full, res


def kernel(**inputs) -> np.ndarray:
    out, _ = run(inputs, trace=False)
    return out


# revision 17
# speedup vs baseline: 1.1680x; 1.1680x over previous
"""Trainium2 Bass kernel for nn_FANPhaseOffsetTransformerLayer.

Full inputs -> full output. Sharding: 8 cores; core c handles batch b=c//4
and sequence-row chunk qc=c%4 (512 rows) of that batch. Each core computes
k/v for its whole batch (4x redundant, zero communication), q only for its
row chunk, attention for its rows over all 16 heads, then Wo/LN1/FAN/LN2
for its rows. Host gathers the 8 row-chunks into the full output.

Matmuls run in bf16 (fp32 PSUM accumulation); softmax scores and LN math
in fp32. Softmax skips the max-subtraction (scores are bounded ~+-4 here,
exp is safe in fp32), and the denominator is produced by a 65th ones-column
appended to v in the PV matmul.
"""

import math

import numpy as np
import ml_dtypes

B, S, D, H, E = 2, 2048, 1024, 16, 64
P_DIM, G_DIM = 256, 512
SC = 512  # rows per core
NCORES = 8
LN_EPS = 1e-5

_bf = ml_dtypes.bfloat16

_prog_cache = {}


def _build_program(gv: float):
    from contextlib import ExitStack

    import concourse.bass as bass
    import concourse.bacc as bacc
    import concourse.tile as tile
    import concourse.mybir as mybir

    f32 = mybir.dt.float32
    bf = mybir.dt.bfloat16
    f32r = mybir.dt.float32r
    AF = mybir.ActivationFunctionType
    ALU = mybir.AluOpType

    nc = bacc.Bacc(
        "TRN2",
        target_bir_lowering=False,
        debug=False,
        enable_asserts=False,
        num_devices=NCORES,
    )

    # ---------------- DRAM I/O ----------------
    d_xT = nc.dram_tensor("xT", [D, S], bf, kind="ExternalInput")
    d_xqT = nc.dram_tensor("xqT", [D, SC], bf, kind="ExternalInput")
    d_xres = nc.dram_tensor("xres", [SC, D], f32, kind="ExternalInput")
    d_wqT = nc.dram_tensor("wqT", [D, D], bf, kind="ExternalInput")
    d_wkT = nc.dram_tensor("wkT", [D, D], bf, kind="ExternalInput")
    d_wvT = nc.dram_tensor("wvT", [D, D], bf, kind="ExternalInput")
    d_woT = nc.dram_tensor("woT", [D, D], bf, kind="ExternalInput")
    d_wpT = nc.dram_tensor("wpT", [D, P_DIM], bf, kind="ExternalInput")
    d_wgT = nc.dram_tensor("wgT", [D, G_DIM], bf, kind="ExternalInput")
    d_bqc = nc.dram_tensor("bqc", [128, 8], f32, kind="ExternalInput")
    d_bkc = nc.dram_tensor("bkc", [128, 8], f32, kind="ExternalInput")
    d_bvr = nc.dram_tensor("bvr", [1, D], bf, kind="ExternalInput")
    d_bor = nc.dram_tensor("bor", [1, D], bf, kind="ExternalInput")
    d_bpr = nc.dram_tensor("bpr", [1, P_DIM], bf, kind="ExternalInput")
    d_bgr = nc.dram_tensor("bgr", [1, G_DIM], bf, kind="ExternalInput")
    d_ln1w = nc.dram_tensor("ln1w", [D], f32, kind="ExternalInput")
    d_ln1b = nc.dram_tensor("ln1b", [D], f32, kind="ExternalInput")
    d_ln2w = nc.dram_tensor("ln2w", [D], f32, kind="ExternalInput")
    d_ln2b = nc.dram_tensor("ln2b", [D], f32, kind="ExternalInput")
    d_offs = nc.dram_tensor("offs", [P_DIM], f32, kind="ExternalInput")
    d_offc = nc.dram_tensor("offc", [P_DIM], f32, kind="ExternalInput")
    d_sel = nc.dram_tensor("sel", [16, 16, 64], f32r, kind="ExternalInput")
    d_ident = nc.dram_tensor("ident", [128, 128], f32, kind="ExternalInput")
    d_out = nc.dram_tensor("out", [SC, D], f32, kind="ExternalOutput")

    def bcast(handle, parts):
        ap_ = handle.ap()
        return bass.AP(
            tensor=ap_.tensor, offset=ap_.offset, ap=[[0, parts]] + list(ap_.ap)
        )

    with tile.TileContext(nc, pool_alloc_mode="queue") as tc:
        with ExitStack() as ctx:
            misc1 = tc.alloc_tile_pool(name="misc1", bufs=1)
            kv = tc.alloc_tile_pool(name="kv", bufs=1, side="right")

            # ------- small constants needed in QKV phase -------
            bqc_sb = misc1.tile([128, 8], f32)
            nc.gpsimd.dma_start(out=bqc_sb, in_=d_bqc.ap())
            bkc_sb = misc1.tile([128, 8], f32)
            nc.gpsimd.dma_start(out=bkc_sb, in_=d_bkc.ap())
            bvr_sb = misc1.tile([1, D], bf)
            nc.gpsimd.dma_start(out=bvr_sb, in_=d_bvr.ap())
            bor_sb = misc1.tile([1, D], bf)
            nc.gpsimd.dma_start(out=bor_sb, in_=d_bor.ap())
            bpr_sb = misc1.tile([1, P_DIM], bf)
            nc.gpsimd.dma_start(out=bpr_sb, in_=d_bpr.ap())
            bgr_sb = misc1.tile([1, G_DIM], bf)
            nc.gpsimd.dma_start(out=bgr_sb, in_=d_bgr.ap())
            ones_row = misc1.tile([1, 128], bf)
            nc.vector.memset(ones_row, 1.0)
            eps_sb = misc1.tile([128, 1], f32)
            nc.vector.memset(eps_sb, LN_EPS)

            # ------- kv-phase persistent tiles -------
            qT_sb = kv.tile([128, 8, SC], bf)
            kT_sb = kv.tile([128, 8, S], bf)
            vaug = kv.tile([128, 16, 16, 65], bf)
            nc.vector.memset(vaug[:, :, :, 64:65], 1.0)

            # ================= QKV + attention (interleaved) =================
            # raw attention output staging (lives through normalize)
            apo = tc.alloc_tile_pool(name="attnp", bufs=1, side="right")
            raw_sb = apo.tile([128, 16, 512], bf)
            den16 = apo.tile([16, 512], bf)
            rec16 = apo.tile([16, 512], f32r)

            with tc.tile_pool(name="qkvw", bufs=1) as qkvw, tc.tile_pool(
                name="xtp", bufs=2
            ) as xtp, tc.tile_pool(name="ppq", bufs=1, space="PSUM") as ppq, tc.tile_pool(
                name="ppa", bufs=1, space="PSUM"
            ) as ppa:
                wq_sb = qkvw.tile([128, 8, D], bf)
                wk_sb = qkvw.tile([128, 8, D], bf)
                wv_sb = qkvw.tile([128, 8, D], bf)
                xq_sb = qkvw.tile([128, 8, SC], bf)
                for kc in range(8):
                    nc.sync.dma_start(
                        out=xq_sb[:, kc, :], in_=d_xqT.ap()[kc * 128 : (kc + 1) * 128, :]
                    )
                    nc.sync.dma_start(
                        out=wk_sb[:, kc, :], in_=d_wkT.ap()[kc * 128 : (kc + 1) * 128, :]
                    )
                for kc in range(8):
                    nc.sync.dma_start(
                        out=wv_sb[:, kc, :], in_=d_wvT.ap()[kc * 128 : (kc + 1) * 128, :]
                    )
                    nc.sync.dma_start(
                        out=wq_sb[:, kc, :], in_=d_wqT.ap()[kc * 128 : (kc + 1) * 128, :]
                    )

                # q^T for own chunk
                for m in range(8):
                    ps = ppq.tile([128, SC], f32, tag="qkvps", bufs=2, name="qps")
                    for kc in range(8):
                        nc.tensor.matmul(
                            ps,
                            lhsT=wq_sb[:, kc, m * 128 : (m + 1) * 128],
                            rhs=xq_sb[:, kc, :],
                            start=(kc == 0),
                            stop=(kc == 7),
                        )
                    nc.vector.tensor_scalar(
                        out=qT_sb[:, m, :],
                        in0=ps,
                        scalar1=bqc_sb[:, m : m + 1],
                        scalar2=None,
                        op0=ALU.add,
                    )

                def emit_kt(tb, xt_sb):
                    for m in range(8):
                        ps = ppq.tile([128, 512], f32, tag="qkvps", bufs=2, name="kps")
                        for kc in range(8):
                            nc.tensor.matmul(
                                ps,
                                lhsT=wk_sb[:, kc, m * 128 : (m + 1) * 128],
                                rhs=xt_sb[:, kc, :],
                                start=(kc == 0),
                                stop=(kc == 7),
                            )
                        nc.vector.tensor_scalar(
                            out=kT_sb[:, m, tb * 512 : (tb + 1) * 512],
                            in0=ps,
                            scalar1=bkc_sb[:, m : m + 1],
                            scalar2=None,
                            op0=ALU.add,
                        )

                def emit_v(tb, xt_sb):
                    for tm in range(4):
                        tcx = tb * 4 + tm
                        for h2 in range(2):
                            ps = ppq.tile(
                                [128, 512], f32, tag="qkvps", bufs=2, name="vps"
                            )
                            for kc in range(8):
                                nc.tensor.matmul(
                                    ps,
                                    lhsT=xt_sb[:, kc, tm * 128 : (tm + 1) * 128],
                                    rhs=wv_sb[:, kc, h2 * 512 : (h2 + 1) * 512],
                                    start=(kc == 0),
                                    stop=False,
                                )
                            nc.tensor.matmul(
                                ps,
                                lhsT=ones_row,
                                rhs=bvr_sb[:, h2 * 512 : (h2 + 1) * 512],
                                start=False,
                                stop=True,
                            )
                            nc.vector.tensor_copy(
                                vaug[:, tcx, h2 * 8 : (h2 + 1) * 8, 0:64], ps
                            )

                def load_xt(tb):
                    xt_sb = xtp.tile([128, 8, 512], bf, tag="xt", name="xt")
                    for kc in range(8):
                        nc.sync.dma_start(
                            out=xt_sb[:, kc, :],
                            in_=d_xT.ap()[
                                kc * 128 : (kc + 1) * 128, tb * 512 : (tb + 1) * 512
                            ],
                        )
                    return xt_sb

                def attn_pair(p, q):
                    opsums = []
                    for j in range(2):
                        op = ppa.tile(
                            [65, 512], f32, tag=f"opsum{j}", bufs=1, name=f"opsum{j}"
                        )
                        opsums.append(op)
                    tbs = range(4 * q, 4 * q + 4)
                    for tb in tbs:
                        ps2 = ppa.tile(
                            [128, 2, 512], f32, tag="ps2", bufs=2, name="ps2"
                        )
                        for j in range(2):
                            off = j * 64
                            nc.tensor.matmul(
                                ps2[:, j, :],
                                lhsT=kT_sb[
                                    off : off + 64, p, tb * 128 : (tb + 1) * 128
                                ],
                                rhs=qT_sb[off : off + 64, p, :],
                                start=True,
                                stop=True,
                            )
                        probs = apo.tile(
                            [128, 2, 512], bf, tag="probs", bufs=3, name="probs"
                        )
                        nc.scalar.activation(
                            out=probs, in_=ps2, func=AF.Exp, scale=1.0 / math.sqrt(E)
                        )
                        for j in range(2):
                            nc.tensor.matmul(
                                opsums[j],
                                lhsT=vaug[:, tb, 2 * p + j, :],
                                rhs=probs[:, j, :],
                                start=(tb == 4 * q),
                                stop=(tb == 4 * q + 3),
                            )
                    for j in range(2):
                        if q == 0:
                            nc.vector.tensor_copy(
                                raw_sb[0:65, 2 * p + j, :], opsums[j]
                            )
                        else:
                            nc.vector.tensor_tensor(
                                out=raw_sb[0:65, 2 * p + j, :],
                                in0=opsums[j],
                                in1=raw_sb[0:65, 2 * p + j, :],
                                op=ALU.add,
                            )

                # quarter-pipelined: kTv(q) then attention over quarter q,
                # with kTv(q+1) interleaved between pairs so PE always has work
                xt0 = load_xt(0)
                emit_kt(0, xt0)
                emit_v(0, xt0)
                for q in range(4):
                    if q < 3:
                        xt_n = load_xt(q + 1)
                        fillers = [
                            lambda: emit_kt(q + 1, xt_n),
                            lambda: emit_v(q + 1, xt_n),
                        ]
                    else:
                        fillers = []
                    for p in range(8):
                        attn_pair(p, q)
                        if q < 3 and p in (1, 4):
                            fillers[0 if p == 1 else 1]()

            # ------- post-phase constants (allocated after QKV pools free) -------
            misc2 = tc.alloc_tile_pool(name="misc2", bufs=1)
            ln1w_bc = misc2.tile([128, D], f32)
            nc.gpsimd.dma_start(out=ln1w_bc, in_=bcast(d_ln1w, 128))
            ln1b_bc = misc2.tile([128, D], f32)
            nc.gpsimd.dma_start(out=ln1b_bc, in_=bcast(d_ln1b, 128))
            ln2w_bc = misc2.tile([128, D], f32)
            nc.gpsimd.dma_start(out=ln2w_bc, in_=bcast(d_ln2w, 128))
            ln2b_bc = misc2.tile([128, D], f32)
            nc.gpsimd.dma_start(out=ln2b_bc, in_=bcast(d_ln2b, 128))
            offs_bc = misc2.tile([128, P_DIM], f32)
            nc.gpsimd.dma_start(out=offs_bc, in_=bcast(d_offs, 128))
            offc_bc = misc2.tile([128, P_DIM], f32)
            nc.gpsimd.dma_start(out=offc_bc, in_=bcast(d_offc, 128))
            sel_sb = misc2.tile([16, 16, 64], f32r)
            nc.gpsimd.dma_start(out=sel_sb, in_=d_sel.ap())
            ident_sb = misc2.tile([128, 128], f32)
            nc.gpsimd.dma_start(out=ident_sb, in_=d_ident.ap())
            xres_sb = misc2.tile([128, 4, D], f32)
            for sc in range(4):
                nc.sync.dma_start(
                    out=xres_sb[:, sc, :], in_=d_xres.ap()[sc * 128 : (sc + 1) * 128, :]
                )
            wo_sb = misc2.tile([128, 8, D], bf)
            for kc in range(8):
                nc.sync.dma_start(
                    out=wo_sb[:, kc, :], in_=d_woT.ap()[kc * 128 : (kc + 1) * 128, :]
                )
            attn_oT = misc2.tile([128, 8, SC], bf)
            odd_sb = misc2.tile([64, 8, 512], bf)

            # softmax denominators -> reciprocals
            nc.sync.dma_start(out=den16, in_=raw_sb[64:65, :, :])
            with nc.allow_low_precision(reason="softmax denominators: f32r rounding is ample"):
                nc.vector.reciprocal(rec16, den16)

            # normalize; even heads direct, odd heads staged then shifted
            with tc.tile_pool(name="ppn", bufs=2, space="PSUM") as ppn:
                for h in range(16):
                    p_, j = h // 2, h % 2
                    div = ppn.tile([64, 512], f32, tag="div", name="div")
                    nc.tensor.matmul(
                        div,
                        lhsT=sel_sb[:, h, :],
                        rhs=rec16,
                        start=True,
                        stop=True,
                    )
                    if j == 0:
                        out_ap = attn_oT[0:64, p_, :]
                    else:
                        out_ap = odd_sb[0:64, p_, :]
                    nc.vector.tensor_tensor(
                        out=out_ap, in0=raw_sb[0:64, h, :], in1=div, op=ALU.mult
                    )
            nc.sync.dma_start(out=attn_oT[64:128, :, :], in_=odd_sb)
            apo.release()
            kv.release()


            # ================= Wo + LN1 + FAN + LN2 =================
            with tc.tile_pool(name="pw", bufs=1) as pw, tc.tile_pool(
                name="post", bufs=2
            ) as po, tc.tile_pool(name="ppp", bufs=2, space="PSUM") as ppp:
                wp_sb = pw.tile([128, 8, P_DIM], bf)
                wg_sb = pw.tile([128, 8, G_DIM], bf)
                for kc in range(8):
                    nc.sync.dma_start(
                        out=wp_sb[:, kc, :], in_=d_wpT.ap()[kc * 128 : (kc + 1) * 128, :]
                    )
                    nc.sync.dma_start(
                        out=wg_sb[:, kc, :], in_=d_wgT.ap()[kc * 128 : (kc + 1) * 128, :]
                    )
                z_sb = pw.tile([128, 4, D], f32, tag="zfan", name="z_sb")
                y_sb = pw.tile([128, 4, D], f32)
                yT_sb = pw.tile([128, 8, SC], bf)
                fan_sb = pw.tile([128, 4, D], f32, tag="zfan", name="fan_sb")
                targ_sb = pw.tile([128, 4, 512], f32)
                g_sb = pw.tile([128, 4, 512], f32)

                def ln_apply(z_ap, w_bc, b_bc, out_ap):
                    stats = po.tile([128, 2, 6], f32, tag="lnst", name="lnst")
                    nc.vector.bn_stats(out=stats[:, 0, :], in_=z_ap[:, 0:512])
                    nc.vector.bn_stats(out=stats[:, 1, :], in_=z_ap[:, 512:1024])
                    mv = po.tile([128, 2], f32, tag="lnmv", name="lnmv")
                    nc.vector.bn_aggr(out=mv, in_=stats)
                    sd = po.tile([128, 2], f32, tag="lnsd", name="lnsd")
                    nc.scalar.activation(
                        out=sd[:, 0:1], in_=mv[:, 1:2], func=AF.Sqrt, bias=eps_sb
                    )
                    nc.vector.reciprocal(sd[:, 1:2], sd[:, 0:1])
                    tmp = po.tile([128, D], f32, tag="lntmp", name="lntmp")
                    nc.vector.tensor_scalar(
                        out=tmp,
                        in0=z_ap,
                        scalar1=mv[:, 0:1],
                        scalar2=sd[:, 1:2],
                        op0=ALU.subtract,
                        op1=ALU.mult,
                    )
                    nc.vector.tensor_tensor(out=tmp, in0=tmp, in1=w_bc, op=ALU.mult)
                    nc.vector.tensor_tensor(out=out_ap, in0=tmp, in1=b_bc, op=ALU.add)

                # Wo projection + residual
                for sc in range(4):
                    for h2 in range(2):
                        ps = ppp.tile([128, 512], f32, tag="wops", name="wops")
                        for kc in range(8):
                            nc.tensor.matmul(
                                ps,
                                lhsT=attn_oT[:, kc, sc * 128 : (sc + 1) * 128],
                                rhs=wo_sb[:, kc, h2 * 512 : (h2 + 1) * 512],
                                start=(kc == 0),
                                stop=False,
                            )
                        nc.tensor.matmul(
                            ps,
                            lhsT=ones_row,
                            rhs=bor_sb[:, h2 * 512 : (h2 + 1) * 512],
                            start=False,
                            stop=True,
                        )
                        nc.vector.tensor_tensor(
                            out=z_sb[:, sc, h2 * 512 : (h2 + 1) * 512],
                            in0=ps,
                            in1=xres_sb[:, sc, h2 * 512 : (h2 + 1) * 512],
                            op=ALU.add,
                        )
                for sc in range(4):
                    ln_apply(z_sb[:, sc, :], ln1w_bc, ln1b_bc, y_sb[:, sc, :])

                # transpose y
                for sc in range(4):
                    for dc in range(8):
                        tp = ppp.tile([128, 128], f32, tag="tp", name="tp")
                        nc.tensor.transpose(
                            tp, y_sb[:, sc, dc * 128 : (dc + 1) * 128], ident_sb
                        )
                        nc.vector.tensor_copy(
                            yT_sb[:, dc, sc * 128 : (sc + 1) * 128], tp
                        )

                # FAN matmuls + activation args
                for sc in range(4):
                    psp = ppp.tile([128, P_DIM], f32, tag="pps", name="pps")
                    for kc in range(8):
                        nc.tensor.matmul(
                            psp,
                            lhsT=yT_sb[:, kc, sc * 128 : (sc + 1) * 128],
                            rhs=wp_sb[:, kc, :],
                            start=(kc == 0),
                            stop=False,
                        )
                    nc.tensor.matmul(
                        psp, lhsT=ones_row, rhs=bpr_sb, start=False, stop=True
                    )
                    nc.vector.tensor_tensor(
                        out=targ_sb[:, sc, 0:256], in0=psp, in1=offs_bc, op=ALU.add
                    )
                    nc.vector.tensor_tensor(
                        out=targ_sb[:, sc, 256:512], in0=psp, in1=offc_bc, op=ALU.add
                    )
                    psg = ppp.tile([128, G_DIM], f32, tag="ppg", name="ppg")
                    for kc in range(8):
                        nc.tensor.matmul(
                            psg,
                            lhsT=yT_sb[:, kc, sc * 128 : (sc + 1) * 128],
                            rhs=wg_sb[:, kc, :],
                            start=(kc == 0),
                            stop=False,
                        )
                    nc.tensor.matmul(
                        psg, lhsT=ones_row, rhs=bgr_sb, start=False, stop=True
                    )
                    nc.vector.tensor_copy(g_sb[:, sc, :], psg)

                # range-reduce sin args to [-pi, pi]:
                # n = round(t/(2pi)) via the fp32 rounding constant; t -= 2pi*n
                RC = 12582912.0  # 1.5 * 2**23
                INV2PI = 1.0 / (2.0 * math.pi)
                for sc in range(4):
                    nred = po.tile([128, 512], f32, tag="nred", name="nred")
                    nc.vector.tensor_scalar(
                        out=nred,
                        in0=targ_sb[:, sc, :],
                        scalar1=INV2PI,
                        scalar2=RC,
                        op0=ALU.mult,
                        op1=ALU.add,
                    )
                    nc.vector.tensor_scalar(
                        out=nred,
                        in0=nred,
                        scalar1=RC,
                        scalar2=None,
                        op0=ALU.subtract,
                    )
                    nc.vector.scalar_tensor_tensor(
                        out=targ_sb[:, sc, :],
                        in0=nred,
                        scalar=-2.0 * math.pi,
                        in1=targ_sb[:, sc, :],
                        op0=ALU.mult,
                        op1=ALU.add,
                    )

                # batched activations (one table set each)
                for sc in range(4):
                    nc.scalar.activation(
                        out=targ_sb[:, sc, :], in_=targ_sb[:, sc, :], func=AF.Sin
                    )
                for sc in range(4):
                    nc.scalar.activation(
                        out=g_sb[:, sc, :], in_=g_sb[:, sc, :], func=AF.Gelu
                    )
                for sc in range(4):
                    nc.vector.tensor_scalar(
                        out=fan_sb[:, sc, 0:512],
                        in0=targ_sb[:, sc, :],
                        scalar1=float(gv),
                        scalar2=None,
                        op0=ALU.mult,
                    )
                    nc.vector.tensor_scalar(
                        out=fan_sb[:, sc, 512:1024],
                        in0=g_sb[:, sc, :],
                        scalar1=float(1.0 - gv),
                        scalar2=None,
                        op0=ALU.mult,
                    )

                # LN2 + output
                for sc in range(4):
                    z2 = po.tile([128, D], f32, tag="z2", name="z2")
                    nc.vector.tensor_tensor(
                        out=z2, in0=y_sb[:, sc, :], in1=fan_sb[:, sc, :], op=ALU.add
                    )
                    outt = po.tile([128, D], f32, tag="outt", name="outt")
                    ln_apply(z2, ln2w_bc, ln2b_bc, outt)
                    nc.sync.dma_start(
                        out=d_out.ap()[sc * 128 : (sc + 1) * 128, :], in_=outt
                    )

            misc2.release()
            misc1.release()

    nc.compile()
    return nc


def _host_inputs(inputs):
    """Build the per-core in_maps (list of 8 dicts) plus baked gate value."""
    f32 = np.float32
    x = np.asarray(inputs["x"], f32)
    Wq = np.asarray(inputs["Wq"], f32)
    Wk = np.asarray(inputs["Wk"], f32)
    Wv = np.asarray(inputs["Wv"], f32)
    Wo = np.asarray(inputs["Wo"], f32)
    Wp = np.asarray(inputs["Wp"], f32)
    Wg = np.asarray(inputs["Wg"], f32)
    bq = np.asarray(inputs["bq"], f32)
    bk = np.asarray(inputs["bk"], f32)
    bv = np.asarray(inputs["bv"], f32)
    bo = np.asarray(inputs["bo"], f32)
    bp = np.asarray(inputs["bp"], f32)
    bg = np.asarray(inputs["bg"], f32)
    offset = np.asarray(inputs["offset"], f32)
    gate = np.asarray(inputs["gate"], f32)
    ln1_w = np.asarray(inputs["ln1_w"], f32)
    ln1_b = np.asarray(inputs["ln1_b"], f32)
    ln2_w = np.asarray(inputs["ln2_w"], f32)
    ln2_b = np.asarray(inputs["ln2_b"], f32)

    gv = float(1.0 / (1.0 + np.exp(-gate[0])))

    sel = np.zeros((16, 16, 64), f32)
    for h in range(16):
        sel[h, h, :] = 1.0
    ident = np.eye(128, dtype=f32)

    shared = {
        "wqT": np.ascontiguousarray(Wq.T).astype(_bf),
        "wkT": np.ascontiguousarray(Wk.T).astype(_bf),
        "wvT": np.ascontiguousarray(Wv.T).astype(_bf),
        "woT": np.ascontiguousarray(Wo.T).astype(_bf),
        "wpT": np.ascontiguousarray(Wp.T).astype(_bf),
        "wgT": np.ascontiguousarray(Wg.T).astype(_bf),
        "bqc": np.ascontiguousarray(bq.reshape(8, 128).T),
        "bkc": np.ascontiguousarray(bk.reshape(8, 128).T),
        "bvr": bv.reshape(1, D).astype(_bf),
        "bor": bo.reshape(1, D).astype(_bf),
        "bpr": bp.reshape(1, P_DIM).astype(_bf),
        "bgr": bg.reshape(1, G_DIM).astype(_bf),
        "ln1w": ln1_w,
        "ln1b": ln1_b,
        "ln2w": ln2_w,
        "ln2b": ln2_b,
        "offs": offset,
        "offc": (np.pi - offset).astype(f32),
        "sel": sel,
        "ident": ident,
    }

    in_maps = []
    for c in range(NCORES):
        b, qc = c // 4, c % 4
        xT_b = np.ascontiguousarray(x[b].T).astype(_bf)
        m = dict(shared)
        m["xT"] = xT_b
        m["xqT"] = np.ascontiguousarray(xT_b[:, qc * SC : (qc + 1) * SC])
        m["xres"] = np.ascontiguousarray(x[b, qc * SC : (qc + 1) * SC])
        in_maps.append(m)
    return in_maps, gv


def run(inputs, trace=False, tmpdir=None):
    """Run the kernel; returns (full_output, BassKernelResults)."""
    from concourse.bass_utils import run_bass_kernel_spmd

    in_maps, gv = _host_inputs(inputs)
    key = round(gv, 9)
    if key not in _prog_cache:
        _prog_cache[key] = _build_program(gv)
    nc = _prog_cache[key]
    res = run_bass_kernel_spmd(
        nc, in_maps, core_ids=list(range(NCORES)), trace=trace, tmpdir=tmpdir
    )
    chunks = [res.results[c]["out"] for c in range(NCORES)]
    full = np.concatenate(chunks, axis=0).reshape(B, S, D).astype(np.float32)
    return full, res


def kernel(**inputs) -> np.ndarray:
    out, _ = run(inputs, trace=False)
    return out


# revision 18
# speedup vs baseline: 1.1945x; 1.0228x over previous
"""Trainium2 Bass kernel for nn_FANPhaseOffsetTransformerLayer.

Full inputs -> full output. Sharding: 8 cores; core c handles batch b=c//4
and sequence-row chunk qc=c%4 (512 rows) of that batch. Each core computes
k/v for its whole batch (4x redundant, zero communication), q only for its
row chunk, attention for its rows over all 16 heads, then Wo/LN1/FAN/LN2
for its rows. Host gathers the 8 row-chunks into the full output.

Matmuls run in bf16 (fp32 PSUM accumulation); softmax scores and LN math
in fp32. Softmax skips the max-subtraction (scores are bounded ~+-4 here,
exp is safe in fp32), and the denominator is produced by a 65th ones-column
appended to v in the PV matmul.
"""

import math

import numpy as np
import ml_dtypes

B, S, D, H, E = 2, 2048, 1024, 16, 64
P_DIM, G_DIM = 256, 512
SC = 512  # rows per core
NCORES = 8
LN_EPS = 1e-5

_bf = ml_dtypes.bfloat16

_prog_cache = {}


def _build_program(gv: float):
    from contextlib import ExitStack

    import concourse.bass as bass
    import concourse.bacc as bacc
    import concourse.tile as tile
    import concourse.mybir as mybir

    f32 = mybir.dt.float32
    bf = mybir.dt.bfloat16
    f32r = mybir.dt.float32r
    AF = mybir.ActivationFunctionType
    ALU = mybir.AluOpType

    nc = bacc.Bacc(
        "TRN2",
        target_bir_lowering=False,
        debug=False,
        enable_asserts=False,
        num_devices=NCORES,
    )

    # ---------------- DRAM I/O ----------------
    d_xT = nc.dram_tensor("xT", [D, S], bf, kind="ExternalInput")
    d_xqT = nc.dram_tensor("xqT", [D, SC], bf, kind="ExternalInput")
    d_xres = nc.dram_tensor("xres", [SC, D], f32, kind="ExternalInput")
    d_wqT = nc.dram_tensor("wqT", [D, D], bf, kind="ExternalInput")
    d_wkT = nc.dram_tensor("wkT", [D, D], bf, kind="ExternalInput")
    d_wvT = nc.dram_tensor("wvT", [D, D], bf, kind="ExternalInput")
    d_woT = nc.dram_tensor("woT", [D, D], bf, kind="ExternalInput")
    d_wpT = nc.dram_tensor("wpT", [D, P_DIM], bf, kind="ExternalInput")
    d_wgT = nc.dram_tensor("wgT", [D, G_DIM], bf, kind="ExternalInput")
    d_bqc = nc.dram_tensor("bqc", [128, 8], f32, kind="ExternalInput")
    d_bkc = nc.dram_tensor("bkc", [128, 8], f32, kind="ExternalInput")
    d_bvf = nc.dram_tensor("bvf", [D], f32, kind="ExternalInput")
    d_bgf = nc.dram_tensor("bgf", [G_DIM], f32, kind="ExternalInput")
    d_bor = nc.dram_tensor("bor", [1, D], bf, kind="ExternalInput")
    d_ln1w = nc.dram_tensor("ln1w", [D], f32, kind="ExternalInput")
    d_ln1b = nc.dram_tensor("ln1b", [D], f32, kind="ExternalInput")
    d_ln2w = nc.dram_tensor("ln2w", [D], f32, kind="ExternalInput")
    d_ln2b = nc.dram_tensor("ln2b", [D], f32, kind="ExternalInput")
    d_offs = nc.dram_tensor("offs", [P_DIM], f32, kind="ExternalInput")
    d_offc = nc.dram_tensor("offc", [P_DIM], f32, kind="ExternalInput")
    d_sel = nc.dram_tensor("sel", [16, 16, 64], f32r, kind="ExternalInput")
    d_ident = nc.dram_tensor("ident", [128, 128], f32, kind="ExternalInput")
    d_out = nc.dram_tensor("out", [SC, D], f32, kind="ExternalOutput")

    def bcast(handle, parts):
        ap_ = handle.ap()
        return bass.AP(
            tensor=ap_.tensor, offset=ap_.offset, ap=[[0, parts]] + list(ap_.ap)
        )

    with tile.TileContext(nc, pool_alloc_mode="queue") as tc:
        with ExitStack() as ctx:
            misc1 = tc.alloc_tile_pool(name="misc1", bufs=1)
            kv = tc.alloc_tile_pool(name="kv", bufs=1, side="right")

            # ------- small constants needed in QKV phase -------
            bqc_sb = misc1.tile([128, 8], f32)
            nc.gpsimd.dma_start(out=bqc_sb, in_=d_bqc.ap())
            bkc_sb = misc1.tile([128, 8], f32)
            nc.gpsimd.dma_start(out=bkc_sb, in_=d_bkc.ap())
            bv_bc = misc1.tile([128, D], f32)
            nc.gpsimd.dma_start(out=bv_bc, in_=bcast(d_bvf, 128))
            bor_sb = misc1.tile([1, D], bf)
            nc.gpsimd.dma_start(out=bor_sb, in_=d_bor.ap())
            ones_row = misc1.tile([1, 128], bf)
            nc.vector.memset(ones_row, 1.0)
            eps_sb = misc1.tile([128, 1], f32)
            nc.vector.memset(eps_sb, LN_EPS)

            # ------- kv-phase persistent tiles -------
            qT_sb = kv.tile([128, 8, SC], bf)
            kT_sb = kv.tile([128, 8, S], bf)
            vaug = kv.tile([128, 16, 16, 65], bf)
            nc.vector.memset(vaug[:, :, :, 64:65], 1.0)

            # ================= QKV + attention (interleaved) =================
            # raw attention output staging (lives through normalize)
            apo = tc.alloc_tile_pool(name="attnp", bufs=1, side="right")
            raw_sb = apo.tile([128, 16, 512], bf)
            den16 = apo.tile([16, 512], bf)
            rec16 = apo.tile([16, 512], f32r)

            with tc.tile_pool(name="qkvw", bufs=1) as qkvw, tc.tile_pool(
                name="xtp", bufs=2
            ) as xtp, tc.tile_pool(name="ppq", bufs=1, space="PSUM") as ppq, tc.tile_pool(
                name="ppa", bufs=1, space="PSUM"
            ) as ppa:
                wq_sb = qkvw.tile([128, 8, D], bf)
                wk_sb = qkvw.tile([128, 8, D], bf)
                wv_sb = qkvw.tile([128, 8, D], bf)
                xq_sb = qkvw.tile([128, 8, SC], bf)
                for kc in range(8):
                    nc.sync.dma_start(
                        out=xq_sb[:, kc, :], in_=d_xqT.ap()[kc * 128 : (kc + 1) * 128, :]
                    )
                    nc.sync.dma_start(
                        out=wk_sb[:, kc, :], in_=d_wkT.ap()[kc * 128 : (kc + 1) * 128, :]
                    )
                for kc in range(8):
                    nc.sync.dma_start(
                        out=wv_sb[:, kc, :], in_=d_wvT.ap()[kc * 128 : (kc + 1) * 128, :]
                    )
                    nc.sync.dma_start(
                        out=wq_sb[:, kc, :], in_=d_wqT.ap()[kc * 128 : (kc + 1) * 128, :]
                    )

                # q^T for own chunk
                for m in range(8):
                    ps = ppq.tile([128, SC], f32, tag="qkvps", bufs=2, name="qps")
                    for kc in range(8):
                        nc.tensor.matmul(
                            ps,
                            lhsT=wq_sb[:, kc, m * 128 : (m + 1) * 128],
                            rhs=xq_sb[:, kc, :],
                            start=(kc == 0),
                            stop=(kc == 7),
                        )
                    nc.vector.tensor_scalar(
                        out=qT_sb[:, m, :],
                        in0=ps,
                        scalar1=bqc_sb[:, m : m + 1],
                        scalar2=None,
                        op0=ALU.add,
                    )

                def emit_kt(tb, xt_sb):
                    for m in range(8):
                        ps = ppq.tile([128, 512], f32, tag="qkvps", bufs=2, name="kps")
                        for kc in range(8):
                            nc.tensor.matmul(
                                ps,
                                lhsT=wk_sb[:, kc, m * 128 : (m + 1) * 128],
                                rhs=xt_sb[:, kc, :],
                                start=(kc == 0),
                                stop=(kc == 7),
                            )
                        nc.vector.tensor_scalar(
                            out=kT_sb[:, m, tb * 512 : (tb + 1) * 512],
                            in0=ps,
                            scalar1=bkc_sb[:, m : m + 1],
                            scalar2=None,
                            op0=ALU.add,
                        )

                def emit_v(tb, xt_sb):
                    for tm in range(4):
                        tcx = tb * 4 + tm
                        for h2 in range(2):
                            ps = ppq.tile(
                                [128, 512], f32, tag="qkvps", bufs=2, name="vps"
                            )
                            for kc in range(8):
                                nc.tensor.matmul(
                                    ps,
                                    lhsT=xt_sb[:, kc, tm * 128 : (tm + 1) * 128],
                                    rhs=wv_sb[:, kc, h2 * 512 : (h2 + 1) * 512],
                                    start=(kc == 0),
                                    stop=(kc == 7),
                                )
                            nc.vector.tensor_tensor(
                                out=vaug[:, tcx, h2 * 8 : (h2 + 1) * 8, 0:64],
                                in0=ps,
                                in1=bv_bc[:, h2 * 512 : (h2 + 1) * 512],
                                op=ALU.add,
                            )

                def load_xt(tb):
                    xt_sb = xtp.tile([128, 8, 512], bf, tag="xt", name="xt")
                    for kc in range(8):
                        nc.sync.dma_start(
                            out=xt_sb[:, kc, :],
                            in_=d_xT.ap()[
                                kc * 128 : (kc + 1) * 128, tb * 512 : (tb + 1) * 512
                            ],
                        )
                    return xt_sb

                def attn_pair(p, q):
                    opsums = []
                    for j in range(2):
                        op = ppa.tile(
                            [65, 512], f32, tag=f"opsum{j}", bufs=1, name=f"opsum{j}"
                        )
                        opsums.append(op)
                    tbs = range(4 * q, 4 * q + 4)
                    for tb in tbs:
                        ps2 = ppa.tile(
                            [128, 2, 512], f32, tag="ps2", bufs=2, name="ps2"
                        )
                        for j in range(2):
                            off = j * 64
                            nc.tensor.matmul(
                                ps2[:, j, :],
                                lhsT=kT_sb[
                                    off : off + 64, p, tb * 128 : (tb + 1) * 128
                                ],
                                rhs=qT_sb[off : off + 64, p, :],
                                start=True,
                                stop=True,
                            )
                        probs = apo.tile(
                            [128, 2, 512], bf, tag="probs", bufs=3, name="probs"
                        )
                        nc.scalar.activation(
                            out=probs, in_=ps2, func=AF.Exp, scale=1.0 / math.sqrt(E)
                        )
                        for j in range(2):
                            nc.tensor.matmul(
                                opsums[j],
                                lhsT=vaug[:, tb, 2 * p + j, :],
                                rhs=probs[:, j, :],
                                start=(tb == 4 * q),
                                stop=(tb == 4 * q + 3),
                            )
                    for j in range(2):
                        if q == 0:
                            nc.vector.tensor_copy(
                                raw_sb[0:65, 2 * p + j, :], opsums[j]
                            )
                        else:
                            nc.vector.tensor_tensor(
                                out=raw_sb[0:65, 2 * p + j, :],
                                in0=opsums[j],
                                in1=raw_sb[0:65, 2 * p + j, :],
                                op=ALU.add,
                            )

                # quarter-pipelined: kTv(q) then attention over quarter q,
                # with kTv(q+1) interleaved between pairs so PE always has work
                xt0 = load_xt(0)
                emit_kt(0, xt0)
                emit_v(0, xt0)
                for q in range(4):
                    if q < 3:
                        xt_n = load_xt(q + 1)
                        fillers = [
                            lambda: emit_kt(q + 1, xt_n),
                            lambda: emit_v(q + 1, xt_n),
                        ]
                    else:
                        fillers = []
                    for p in range(8):
                        attn_pair(p, q)
                        if q < 3 and p in (1, 4):
                            fillers[0 if p == 1 else 1]()

            # ------- post-phase constants (allocated after QKV pools free) -------
            misc2 = tc.alloc_tile_pool(name="misc2", bufs=1)
            ln1w_bc = misc2.tile([128, D], f32)
            nc.gpsimd.dma_start(out=ln1w_bc, in_=bcast(d_ln1w, 128))
            ln1b_bc = misc2.tile([128, D], f32)
            nc.gpsimd.dma_start(out=ln1b_bc, in_=bcast(d_ln1b, 128))
            ln2w_bc = misc2.tile([128, D], f32)
            nc.gpsimd.dma_start(out=ln2w_bc, in_=bcast(d_ln2w, 128))
            ln2b_bc = misc2.tile([128, D], f32)
            nc.gpsimd.dma_start(out=ln2b_bc, in_=bcast(d_ln2b, 128))
            bg_bc = misc2.tile([128, G_DIM], f32)
            nc.gpsimd.dma_start(out=bg_bc, in_=bcast(d_bgf, 128))
            offs_bc = misc2.tile([128, P_DIM], f32)
            nc.gpsimd.dma_start(out=offs_bc, in_=bcast(d_offs, 128))
            offc_bc = misc2.tile([128, P_DIM], f32)
            nc.gpsimd.dma_start(out=offc_bc, in_=bcast(d_offc, 128))
            sel_sb = misc2.tile([16, 16, 64], f32r)
            nc.gpsimd.dma_start(out=sel_sb, in_=d_sel.ap())
            ident_sb = misc2.tile([128, 128], f32)
            nc.gpsimd.dma_start(out=ident_sb, in_=d_ident.ap())
            xres_sb = misc2.tile([128, 4, D], f32)
            for sc in range(4):
                nc.sync.dma_start(
                    out=xres_sb[:, sc, :], in_=d_xres.ap()[sc * 128 : (sc + 1) * 128, :]
                )
            wo_sb = misc2.tile([128, 8, D], bf)
            for kc in range(8):
                nc.sync.dma_start(
                    out=wo_sb[:, kc, :], in_=d_woT.ap()[kc * 128 : (kc + 1) * 128, :]
                )
            attn_oT = misc2.tile([128, 8, SC], bf)
            odd_sb = misc2.tile([64, 8, 512], bf)

            # softmax denominators -> reciprocals
            nc.sync.dma_start(out=den16, in_=raw_sb[64:65, :, :])
            with nc.allow_low_precision(reason="softmax denominators: f32r rounding is ample"):
                nc.vector.reciprocal(rec16, den16)

            # normalize; even heads direct, odd heads staged then shifted
            with tc.tile_pool(name="ppn", bufs=2, space="PSUM") as ppn:
                for h in range(16):
                    p_, j = h // 2, h % 2
                    div = ppn.tile([64, 512], f32, tag="div", name="div")
                    nc.tensor.matmul(
                        div,
                        lhsT=sel_sb[:, h, :],
                        rhs=rec16,
                        start=True,
                        stop=True,
                    )
                    if j == 0:
                        out_ap = attn_oT[0:64, p_, :]
                    else:
                        out_ap = odd_sb[0:64, p_, :]
                    nc.vector.tensor_tensor(
                        out=out_ap, in0=raw_sb[0:64, h, :], in1=div, op=ALU.mult
                    )
            nc.sync.dma_start(out=attn_oT[64:128, :, :], in_=odd_sb)
            apo.release()
            kv.release()


            # ================= Wo + LN1 + FAN + LN2 =================
            with tc.tile_pool(name="pw", bufs=1) as pw, tc.tile_pool(
                name="post", bufs=2
            ) as po, tc.tile_pool(name="ppp", bufs=2, space="PSUM") as ppp:
                wp_sb = pw.tile([128, 8, P_DIM], bf)
                wg_sb = pw.tile([128, 8, G_DIM], bf)
                for kc in range(8):
                    nc.sync.dma_start(
                        out=wp_sb[:, kc, :], in_=d_wpT.ap()[kc * 128 : (kc + 1) * 128, :]
                    )
                    nc.sync.dma_start(
                        out=wg_sb[:, kc, :], in_=d_wgT.ap()[kc * 128 : (kc + 1) * 128, :]
                    )
                z_sb = pw.tile([128, 4, D], f32, tag="zfan", name="z_sb")
                y_sb = pw.tile([128, 4, D], f32)
                yT_sb = pw.tile([128, 8, SC], bf)
                fan_sb = pw.tile([128, 4, D], f32, tag="zfan", name="fan_sb")
                targ_sb = pw.tile([128, 4, 512], f32)
                g_sb = pw.tile([128, 4, 512], f32)

                def ln_apply(z_ap, w_bc, b_bc, out_ap):
                    stats = po.tile([128, 2, 6], f32, tag="lnst", name="lnst")
                    nc.vector.bn_stats(out=stats[:, 0, :], in_=z_ap[:, 0:512])
                    nc.vector.bn_stats(out=stats[:, 1, :], in_=z_ap[:, 512:1024])
                    mv = po.tile([128, 2], f32, tag="lnmv", name="lnmv")
                    nc.vector.bn_aggr(out=mv, in_=stats)
                    sd = po.tile([128, 2], f32, tag="lnsd", name="lnsd")
                    nc.scalar.activation(
                        out=sd[:, 0:1], in_=mv[:, 1:2], func=AF.Sqrt, bias=eps_sb
                    )
                    nc.vector.reciprocal(sd[:, 1:2], sd[:, 0:1])
                    tmp = po.tile([128, D], f32, tag="lntmp", name="lntmp")
                    nc.vector.tensor_scalar(
                        out=tmp,
                        in0=z_ap,
                        scalar1=mv[:, 0:1],
                        scalar2=sd[:, 1:2],
                        op0=ALU.subtract,
                        op1=ALU.mult,
                    )
                    nc.vector.tensor_tensor(out=tmp, in0=tmp, in1=w_bc, op=ALU.mult)
                    nc.vector.tensor_tensor(out=out_ap, in0=tmp, in1=b_bc, op=ALU.add)

                # Wo projection + residual
                for sc in range(4):
                    for h2 in range(2):
                        ps = ppp.tile([128, 512], f32, tag="wops", name="wops")
                        for kc in range(8):
                            nc.tensor.matmul(
                                ps,
                                lhsT=attn_oT[:, kc, sc * 128 : (sc + 1) * 128],
                                rhs=wo_sb[:, kc, h2 * 512 : (h2 + 1) * 512],
                                start=(kc == 0),
                                stop=False,
                            )
                        nc.tensor.matmul(
                            ps,
                            lhsT=ones_row,
                            rhs=bor_sb[:, h2 * 512 : (h2 + 1) * 512],
                            start=False,
                            stop=True,
                        )
                        nc.vector.tensor_tensor(
                            out=z_sb[:, sc, h2 * 512 : (h2 + 1) * 512],
                            in0=ps,
                            in1=xres_sb[:, sc, h2 * 512 : (h2 + 1) * 512],
                            op=ALU.add,
                        )
                for sc in range(4):
                    ln_apply(z_sb[:, sc, :], ln1w_bc, ln1b_bc, y_sb[:, sc, :])

                # transpose y
                for sc in range(4):
                    for dc in range(8):
                        tp = ppp.tile([128, 128], f32, tag="tp", name="tp")
                        nc.tensor.transpose(
                            tp, y_sb[:, sc, dc * 128 : (dc + 1) * 128], ident_sb
                        )
                        nc.vector.tensor_copy(
                            yT_sb[:, dc, sc * 128 : (sc + 1) * 128], tp
                        )

                # FAN matmuls + activation args
                for sc in range(4):
                    psp = ppp.tile([128, P_DIM], f32, tag="pps", name="pps")
                    for kc in range(8):
                        nc.tensor.matmul(
                            psp,
                            lhsT=yT_sb[:, kc, sc * 128 : (sc + 1) * 128],
                            rhs=wp_sb[:, kc, :],
                            start=(kc == 0),
                            stop=(kc == 7),
                        )
                    nc.vector.tensor_tensor(
                        out=targ_sb[:, sc, 0:256], in0=psp, in1=offs_bc, op=ALU.add
                    )
                    nc.vector.tensor_tensor(
                        out=targ_sb[:, sc, 256:512], in0=psp, in1=offc_bc, op=ALU.add
                    )
                    psg = ppp.tile([128, G_DIM], f32, tag="ppg", name="ppg")
                    for kc in range(8):
                        nc.tensor.matmul(
                            psg,
                            lhsT=yT_sb[:, kc, sc * 128 : (sc + 1) * 128],
                            rhs=wg_sb[:, kc, :],
                            start=(kc == 0),
                            stop=(kc == 7),
                        )
                    nc.vector.tensor_tensor(
                        out=g_sb[:, sc, :], in0=psg, in1=bg_bc, op=ALU.add
                    )

                # range-reduce sin args to [-pi, pi]:
                # n = round(t/(2pi)) via the fp32 rounding constant; t -= 2pi*n
                RC = 12582912.0  # 1.5 * 2**23
                INV2PI = 1.0 / (2.0 * math.pi)
                for sc in range(4):
                    nred = po.tile([128, 512], f32, tag="nred", name="nred")
                    nc.vector.tensor_scalar(
                        out=nred,
                        in0=targ_sb[:, sc, :],
                        scalar1=INV2PI,
                        scalar2=RC,
                        op0=ALU.mult,
                        op1=ALU.add,
                    )
                    nc.vector.tensor_scalar(
                        out=nred,
                        in0=nred,
                        scalar1=RC,
                        scalar2=None,
                        op0=ALU.subtract,
                    )
                    nc.vector.scalar_tensor_tensor(
                        out=targ_sb[:, sc, :],
                        in0=nred,
                        scalar=-2.0 * math.pi,
                        in1=targ_sb[:, sc, :],
                        op0=ALU.mult,
                        op1=ALU.add,
                    )

                # batched activations (one table set each)
                for sc in range(4):
                    nc.scalar.activation(
                        out=targ_sb[:, sc, :], in_=targ_sb[:, sc, :], func=AF.Sin
                    )
                for sc in range(4):
                    nc.scalar.activation(
                        out=g_sb[:, sc, :], in_=g_sb[:, sc, :], func=AF.Gelu
                    )
                for sc in range(4):
                    nc.vector.tensor_scalar(
                        out=fan_sb[:, sc, 0:512],
                        in0=targ_sb[:, sc, :],
                        scalar1=float(gv),
                        scalar2=None,
                        op0=ALU.mult,
                    )
                    nc.vector.tensor_scalar(
                        out=fan_sb[:, sc, 512:1024],
                        in0=g_sb[:, sc, :],
                        scalar1=float(1.0 - gv),
                        scalar2=None,
                        op0=ALU.mult,
                    )

                # LN2 + output
                for sc in range(4):
                    z2 = po.tile([128, D], f32, tag="z2", name="z2")
                    nc.vector.tensor_tensor(
                        out=z2, in0=y_sb[:, sc, :], in1=fan_sb[:, sc, :], op=ALU.add
                    )
                    outt = po.tile([128, D], f32, tag="outt", name="outt")
                    ln_apply(z2, ln2w_bc, ln2b_bc, outt)
                    nc.sync.dma_start(
                        out=d_out.ap()[sc * 128 : (sc + 1) * 128, :], in_=outt
                    )

            misc2.release()
            misc1.release()

    nc.compile()
    return nc


def _host_inputs(inputs):
    """Build the per-core in_maps (list of 8 dicts) plus baked gate value."""
    f32 = np.float32
    x = np.asarray(inputs["x"], f32)
    Wq = np.asarray(inputs["Wq"], f32)
    Wk = np.asarray(inputs["Wk"], f32)
    Wv = np.asarray(inputs["Wv"], f32)
    Wo = np.asarray(inputs["Wo"], f32)
    Wp = np.asarray(inputs["Wp"], f32)
    Wg = np.asarray(inputs["Wg"], f32)
    bq = np.asarray(inputs["bq"], f32)
    bk = np.asarray(inputs["bk"], f32)
    bv = np.asarray(inputs["bv"], f32)
    bo = np.asarray(inputs["bo"], f32)
    bp = np.asarray(inputs["bp"], f32)
    bg = np.asarray(inputs["bg"], f32)
    offset = np.asarray(inputs["offset"], f32)
    gate = np.asarray(inputs["gate"], f32)
    ln1_w = np.asarray(inputs["ln1_w"], f32)
    ln1_b = np.asarray(inputs["ln1_b"], f32)
    ln2_w = np.asarray(inputs["ln2_w"], f32)
    ln2_b = np.asarray(inputs["ln2_b"], f32)

    gv = float(1.0 / (1.0 + np.exp(-gate[0])))

    sel = np.zeros((16, 16, 64), f32)
    for h in range(16):
        sel[h, h, :] = 1.0
    ident = np.eye(128, dtype=f32)

    shared = {
        "wqT": np.ascontiguousarray(Wq.T).astype(_bf),
        "wkT": np.ascontiguousarray(Wk.T).astype(_bf),
        "wvT": np.ascontiguousarray(Wv.T).astype(_bf),
        "woT": np.ascontiguousarray(Wo.T).astype(_bf),
        "wpT": np.ascontiguousarray(Wp.T).astype(_bf),
        "wgT": np.ascontiguousarray(Wg.T).astype(_bf),
        "bqc": np.ascontiguousarray(bq.reshape(8, 128).T),
        "bkc": np.ascontiguousarray(bk.reshape(8, 128).T),
        "bvf": bv,
        "bgf": bg,
        "bor": bo.reshape(1, D).astype(_bf),
        "ln1w": ln1_w,
        "ln1b": ln1_b,
        "ln2w": ln2_w,
        "ln2b": ln2_b,
        "offs": (offset + bp).astype(f32),
        "offc": (np.pi - offset + bp).astype(f32),
        "sel": sel,
        "ident": ident,
    }

    in_maps = []
    for c in range(NCORES):
        b, qc = c // 4, c % 4
        xT_b = np.ascontiguousarray(x[b].T).astype(_bf)
        m = dict(shared)
        m["xT"] = xT_b
        m["xqT"] = np.ascontiguousarray(xT_b[:, qc * SC : (qc + 1) * SC])
        m["xres"] = np.ascontiguousarray(x[b, qc * SC : (qc + 1) * SC])
        in_maps.append(m)
    return in_maps, gv


def run(inputs, trace=False, tmpdir=None):
    """Run the kernel; returns (full_output, BassKernelResults)."""
    from concourse.bass_utils import run_bass_kernel_spmd

    in_maps, gv = _host_inputs(inputs)
    key = round(gv, 9)
    if key not in _prog_cache:
        _prog_cache[key] = _build_program(gv)
    nc = _prog_cache[key]
    res = run_bass_kernel_spmd(
        nc, in_maps, core_ids=list(range(NCORES)), trace=trace, tmpdir=tmpdir
    )
    chunks = [res.results[c]["out"] for c in range(NCORES)]
    full = np.concatenate(chunks, axis=0).reshape(B, S, D).astype(np.float32)
    return full, res


def kernel(**inputs) -> np.ndarray:
    out, _ = run(inputs, trace=False)
    return out
